# revision 1
# baseline (speedup 1.0000x reference)
"""Trainium2 Bass kernel: 12-layer transformer encoder forward pass.

Strategy: pure data-parallel over batch — 8 NeuronCores x 1 batch element.
No collectives. Per core, activations are kept feature-on-partition
("f-layout", i.e. transposed: [D, S]) so every matmul consumes weights
as stored with zero on-chip transposes:

  out^T[M=dout, N=tok] = lhsT[K=din, M=dout].T @ rhs[K=din, N=tok]

All matmuls run in float32r (full PE rate for moving dim >= 256, ~1.5e-4
rel err). LayerNorm stats use ones-vector matmul partition reductions;
per-token scale/shift rows are broadcast to 128 partitions with K=1
matmuls. Softmax skips max-subtraction (scores are O(+-10) here) and is
computed directly in transposed layout: S^T = k_h^T.T-matmul, exp on the
scalar engine, and the per-query sum(exp) obtained for free by appending
a ones-column to V in the P^T@V matmul (row 64 of the 65-row output).
LN scale/bias, the 1/sqrt(hd) score scale, V/Q/K biases and the GELU 0.5
factor are all folded into the weights host-side.
"""

import sys

sys.path.insert(0, "/opt/trn_rl_repo")

import numpy as np

L = 12
D = 768
H = 12
F = 3072
B = 8
S = 784
HD = 64
DT = D // 128  # 6 feature tiles
FT = F // 128  # 24 mlp tiles
TT = (S + 127) // 128  # 7 token tiles, last has 16 rows
EPS = 1e-6
SPL = ((0, 512), (512, 272))  # token splits (bank-aligned in a 2-bank psum tile)
VSPL = ((0, 512), (512, 256))  # dout splits for the v projection
MH = ((0, 392), (392, 392))  # mlp token halves
INV_SQRT2 = 0.7071067811865476

_CACHE = {}


def _tok(t):
    """(offset, rows) of token tile t."""
    return t * 128, (128 if t < TT - 1 else S - 128 * (TT - 1))


def _build(n_layers=L):
    import concourse.mybir as mybir
    from concourse import bacc
    from concourse.tile import TileContext

    f32 = mybir.dt.float32
    f32r = mybir.dt.float32r
    AF = mybir.ActivationFunctionType
    OP = mybir.AluOpType

    def r(ap):
        return ap.bitcast(f32r)

    nc = bacc.Bacc("TRN2", target_bir_lowering=False)

    xT = nc.dram_tensor("xT", [D, S], f32, kind="ExternalInput")
    wq = nc.dram_tensor("wq", [n_layers, DT, 128, D], f32, kind="ExternalInput")
    wk = nc.dram_tensor("wk", [n_layers, DT, 128, D], f32, kind="ExternalInput")
    wv = nc.dram_tensor("wv", [n_layers, D, D], f32, kind="ExternalInput")
    wo = nc.dram_tensor("wo", [n_layers, DT, 128, D], f32, kind="ExternalInput")
    w1 = nc.dram_tensor("w1", [n_layers, FT, 128, D], f32, kind="ExternalInput")
    w2 = nc.dram_tensor("w2", [n_layers, DT, 128, F], f32, kind="ExternalInput")
    bq = nc.dram_tensor("bq", [n_layers, 128, DT], f32, kind="ExternalInput")
    bk = nc.dram_tensor("bk", [n_layers, 128, DT], f32, kind="ExternalInput")
    bo = nc.dram_tensor("bo", [n_layers, 128, DT], f32, kind="ExternalInput")
    b2 = nc.dram_tensor("b2", [n_layers, 128, DT], f32, kind="ExternalInput")
    b1r = nc.dram_tensor("b1r", [n_layers, F], f32, kind="ExternalInput")
    lnf = nc.dram_tensor("lnf", [128, DT, 2], f32, kind="ExternalInput")
    onesd = nc.dram_tensor("onesd", [128, S], f32, kind="ExternalInput")
    outT = nc.dram_tensor("outT", [D, S], f32, kind="ExternalOutput")

    with TileContext(nc) as tc:
        from contextlib import ExitStack

        with ExitStack() as ctx:
            perm = ctx.enter_context(tc.tile_pool(name="perm", bufs=1))
            acts = ctx.enter_context(tc.tile_pool(name="acts", bufs=1))
            qkp = ctx.enter_context(tc.tile_pool(name="qk", bufs=1))
            ep = ctx.enter_context(tc.tile_pool(name="ep", bufs=2))
            gp = ctx.enter_context(tc.tile_pool(name="gp", bufs=1))
            scr = ctx.enter_context(tc.tile_pool(name="scr", bufs=3))
            wap = ctx.enter_context(tc.tile_pool(name="wap", bufs=2))
            wvp = ctx.enter_context(tc.tile_pool(name="wvp", bufs=1))
            w1p = ctx.enter_context(tc.tile_pool(name="w1p", bufs=2))
            w2p = ctx.enter_context(tc.tile_pool(name="w2p", bufs=2))
            bp = ctx.enter_context(tc.tile_pool(name="bp", bufs=2))
            pbig = ctx.enter_context(tc.tile_pool(name="pbig", bufs=2, space="PSUM"))
            pctx = ctx.enter_context(tc.tile_pool(name="pctx", bufs=2, space="PSUM"))

            # persistent tiles
            h = [perm.tile([128, S], f32, tag=f"h{d}", name=f"h{d}") for d in range(DT)]
            v = [perm.tile([128, H * 65], f32, tag=f"v{t}", name=f"v{t}") for t in range(TT)]
            cx = [perm.tile([128, S], f32, tag=f"c{d}", name=f"c{d}") for d in range(DT)]
            ones = perm.tile([128, S], f32, tag="ones")
            stat = perm.tile([128, S], f32, tag="stat")
            statb = perm.tile([128, S], f32, tag="statb")
            statc = perm.tile([128, S], f32, tag="statc")
            statd = perm.tile([128, S], f32, tag="statd")
            lnft = perm.tile([128, DT * 2], f32, tag="lnft")

            nc.sync.dma_start(r(ones), r(onesd[:, :]))
            nc.sync.dma_start(lnft[:], lnf[:, :, :])
            for d in range(DT):
                nc.sync.dma_start(r(h[d]), r(xT[d * 128 : (d + 1) * 128, :]))
            # ones columns of the augmented V (column 64 of each head's 65)
            for t in range(TT):
                vap = v[t][:].rearrange("p (a c) -> p a c", c=65)[:, :, 64:65]
                nc.vector.tensor_copy(r(vap), r(ones[:, 0:H]))

            ones_col = ones[:, 0:1]  # lhsT [K=128, M=1] for partition sums
            ones_row = ones[0:1, 0:128]  # lhsT [K=1, M<=128] for broadcasts
            ones_S = ones[0:1, :]  # rhs [1, S] for the +b1 rank-1 matmul

            def emit_ln(src, dst, final=False):
                """dst[d] = (src[d] - mean)*rsqrt(var+eps) feature-on-partition.
                If final, also apply lnf scale/bias per partition."""
                # E[x] and E[x^2] rows via ones-matmul partition reduction
                ps_s = pbig.tile([128, S], f32, tag="pb", name="pb")
                ps_q = pbig.tile([128, S], f32, tag="pb", name="pb")
                sq = []
                for d in range(DT):
                    sqt = scr.tile([128, S], f32, tag="sq")
                    nc.vector.tensor_tensor(r(sqt), src[d][:], src[d][:], OP.mult)
                    sq.append(sqt)
                for d in range(DT):
                    for o, n in SPL:
                        nc.tensor.matmul(
                            ps_s[0:1, o : o + n],
                            r(ones_col),
                            r(src[d][:, o : o + n]),
                            start=(d == 0),
                            stop=(d == DT - 1),
                        )
                for d in range(DT):
                    for o, n in SPL:
                        nc.tensor.matmul(
                            ps_q[0:1, o : o + n],
                            r(ones_col),
                            r(sq[d][:, o : o + n]),
                            start=(d == 0),
                            stop=(d == DT - 1),
                        )
                # Engine rule: both-SBUF-input DVE ops need equal base
                # partitions; rows that are matmul rhs need base in {0,32,64}
                # matching the ones-lhsT row. Chain below keeps PSUM as one
                # input wherever rows at different bases combine.
                # region discipline: statb[64] only ever gets f32r writes
                # (a_row is fp32r-matmul rhs); plain-f32 scratch lives in stat.
                # custom-DVE ops (reciprocal_approx_fast) require base
                # partition 0 and equal in/out bases; keep t/u rows at 0.
                s_row = stat[96:97, :]
                sq_row = statd[96:97, :]
                t_row = stat[0:1, :]
                u_row = statd[0:1, :]
                a_row = statb[64:65, :]
                c_row = statb[32:33, :]
                # s = sum(x) ; t = ps_q*D - s^2 + eps*D^2 = (var+eps)*D^2
                # a = sqrt(D^2 * recip(t)) = 1/sqrt(var+eps)  (scale folded
                # into the ACT sqrt; recip_approx_fast is ~5x faster than
                # the exact DVE reciprocal and 18 bits is plenty here)
                nc.scalar.activation(s_row, ps_s[0:1, :], AF.Copy)
                nc.vector.tensor_tensor(sq_row, s_row, s_row, OP.mult)
                nc.vector.scalar_tensor_tensor(
                    t_row, ps_q[0:1, :], float(D), sq_row, OP.mult, OP.subtract
                )
                nc.vector.tensor_scalar_add(t_row, t_row, EPS * D * D)
                nc.vector.reciprocal_approx_fast(out=u_row, in_=t_row)
                nc.scalar.activation(r(a_row), u_row, AF.Sqrt, scale=float(D * D))
                nc.vector.scalar_tensor_tensor(
                    r(c_row), ps_s[0:1, :], -1.0 / D, a_row, OP.mult, OP.mult
                )
                ps_a = pbig.tile([128, S], f32, tag="pb", name="pb")
                ps_c = pbig.tile([128, S], f32, tag="pb", name="pb")
                for o, n in SPL:
                    nc.tensor.matmul(
                        ps_a[:, o : o + n], r(ones[64:65, 0:128]), r(a_row[:, o : o + n]),
                        start=True, stop=True,
                    )
                    nc.tensor.matmul(
                        ps_c[:, o : o + n], r(ones[32:33, 0:128]), r(c_row[:, o : o + n]),
                        start=True, stop=True,
                    )
                for d in range(DT):
                    nc.vector.tensor_tensor(r(dst[d][:]), src[d][:], ps_a[:, :], OP.mult)
                    nc.vector.tensor_tensor(r(dst[d][:]), dst[d][:], ps_c[:, :], OP.add)
                    if final:
                        nc.vector.tensor_scalar(
                            out=dst[d][:],
                            in0=dst[d][:],
                            scalar1=lnft[:, 2 * d : 2 * d + 1],
                            scalar2=lnft[:, 2 * d + 1 : 2 * d + 2],
                            op0=OP.mult,
                            op1=OP.add,
                        )

            for lay in range(n_layers):
                # ---- LN1 ----
                y = [acts.tile([128, S], f32, tag=f"y{d}", name=f"yl{d}") for d in range(DT)]
                emit_ln(h, y)

                # ---- biases for this layer ----
                bqt = bp.tile([128, DT], f32, tag="bq")
                bkt = bp.tile([128, DT], f32, tag="bk")
                bot = bp.tile([128, DT], f32, tag="bo")
                b2t = bp.tile([128, DT], f32, tag="b2")
                nc.sync.dma_start(bqt[:], bq[lay])
                nc.sync.dma_start(bkt[:], bk[lay])
                nc.sync.dma_start(bot[:], bo[lay])
                nc.sync.dma_start(b2t[:], b2[lay])

                # ---- V projection (token-on-partition, into augmented V) ----
                for o, n in VSPL:
                    wvs = []
                    for kd in range(DT):
                        wvt = wvp.tile([128, 512], f32, tag=f"wv{kd}")
                        nc.sync.dma_start(
                            r(wvt[:, 0:n]), r(wv[lay, kd * 128 : (kd + 1) * 128, o : o + n])
                        )
                        wvs.append(wvt)
                    a0 = o // 64
                    na = n // 64
                    for t in range(TT):
                        to, tm = _tok(t)
                        pv = pbig.tile([128, S], f32, tag="pb", name="pb")
                        for kd in range(DT):
                            nc.tensor.matmul(
                                pv[0:tm, 0:n],
                                r(y[kd][:, to : to + tm]),
                                r(wvs[kd][:, 0:n]),
                                start=(kd == 0),
                                stop=(kd == DT - 1),
                            )
                        dst = v[t][:].rearrange("p (a c) -> p a c", c=65)[
                            0:tm, a0 : a0 + na, 0:64
                        ]
                        srcv = pv[0:tm, 0:n].rearrange("p (a c) -> p a c", c=64)
                        nc.vector.tensor_copy(r(dst), srcv)

                # ---- attention head pairs ----
                for j in range(DT):
                    qt = qkp.tile([128, S], f32, tag="q")
                    kt = qkp.tile([128, S], f32, tag="k")
                    for dst, wsrc, bt in ((qt, wq, bqt), (kt, wk, bkt)):
                        wt = wap.tile([128, D], f32, tag="wa")
                        nc.sync.dma_start(r(wt), r(wsrc[lay, j]))
                        pq = pbig.tile([128, S], f32, tag="pb", name="pb")
                        for o, n in SPL:
                            for kd in range(DT):
                                nc.tensor.matmul(
                                    pq[:, o : o + n],
                                    r(wt[:, kd * 128 : (kd + 1) * 128]),
                                    r(y[kd][:, o : o + n]),
                                    start=(kd == 0),
                                    stop=(kd == DT - 1),
                                )
                        nc.scalar.activation(
                            r(dst[:]), pq[:, :], AF.Identity, bias=bt[:, j : j + 1]
                        )
                    psC = [pctx.tile([128, S], f32, tag="pc", name=f"psC{j}_{i}") for i in range(2)]
                    for t in range(TT):
                        to, tm = _tok(t)
                        pss = []
                        for hh in range(2):
                            po = hh * 64
                            ps = pbig.tile([128, S], f32, tag="pb", name="pb")
                            pss.append(ps)
                            for o, n in SPL:
                                nc.tensor.matmul(
                                    ps[0:tm, o : o + n],
                                    r(kt[po : po + 64, to : to + tm]),
                                    r(qt[po : po + 64, o : o + n]),
                                    start=True,
                                    stop=True,
                                )
                        for hh in range(2):
                            hd_ = 2 * j + hh
                            e = ep.tile([128, S], f32, tag="e")
                            nc.scalar.activation(r(e[0:tm, :]), pss[hh][0:tm, :], AF.Exp)
                            for o, n in SPL:
                                nc.tensor.matmul(
                                    psC[hh][0:65, o : o + n],
                                    r(v[t][0:tm, hd_ * 65 : hd_ * 65 + 65]),
                                    r(e[0:tm, o : o + n]),
                                    start=(t == 0),
                                    stop=(t == TT - 1),
                                )
                    for hh in range(2):
                        hd_ = 2 * j + hh
                        po = hh * 64
                        rp = 0 if hh == 0 else 32
                        # broadcast sum(exp) (row 64) to 64 partitions, take a
                        # wide fast reciprocal, then scale the raw context.
                        # broadcast sum(exp) to 64 partitions, wide fast
                        # reciprocal (psum->psum, base 0), then scale ctx.
                        serow = statc[rp : rp + 1, :]
                        nc.scalar.activation(r(serow), psC[hh][64:65, :], AF.Copy)
                        pb = pbig.tile([128, S], f32, tag="pb", name="pb")
                        for o, n in SPL:
                            nc.tensor.matmul(
                                pb[0:64, o : o + n],
                                r(ones[rp : rp + 1, 0:64]),
                                r(serow[:, o : o + n]),
                                start=True,
                                stop=True,
                            )
                        pbr = pbig.tile([128, S], f32, tag="pb", name="pbr")
                        nc.vector.reciprocal_approx_fast(
                            out=pbr[0:64, :], in_=pb[0:64, :]
                        )
                        nc.scalar.activation(
                            r(cx[j][po : po + 64, :]), psC[hh][0:64, :], AF.Copy
                        )
                        nc.vector.tensor_tensor(
                            r(cx[j][po : po + 64, :]),
                            cx[j][po : po + 64, :],
                            pbr[0:64, :],
                            OP.mult,
                        )

                # ---- O projection + residual ----
                for m in range(DT):
                    wt = wap.tile([128, D], f32, tag="wa")
                    nc.sync.dma_start(r(wt), r(wo[lay, m]))
                    po_ = pbig.tile([128, S], f32, tag="pb", name="pb")
                    for o, n in SPL:
                        for kd in range(DT):
                            nc.tensor.matmul(
                                po_[:, o : o + n],
                                r(wt[:, kd * 128 : (kd + 1) * 128]),
                                r(cx[kd][:, o : o + n]),
                                start=(kd == 0),
                                stop=(kd == DT - 1),
                            )
                    nc.vector.scalar_tensor_tensor(
                        r(h[m][:]), po_[:, :], bot[:, m : m + 1], h[m][:], OP.add, OP.add
                    )

                # ---- LN2 ----
                y2 = [acts.tile([128, S], f32, tag=f"y{d}", name=f"y2l{d}") for d in range(DT)]
                emit_ln(h, y2)

                # ---- MLP (two token halves; weights streamed per half) ----
                for ho, hn in MH:
                    g = []
                    for m in range(FT):
                        w1t = w1p.tile([128, D], f32, tag="w1")
                        nc.sync.dma_start(r(w1t), r(w1[lay, m]))
                        pz = pbig.tile([128, S], f32, tag="pb", name="pb")
                        for kd in range(DT):
                            nc.tensor.matmul(
                                pz[:, 0:hn],
                                r(w1t[:, kd * 128 : (kd + 1) * 128]),
                                r(y2[kd][:, ho : ho + hn]),
                                start=(kd == 0),
                                stop=False,
                            )
                        b1m = bp.tile([1, 128], f32, tag="b1")
                        nc.sync.dma_start(
                            r(b1m), r(b1r[lay : lay + 1, m * 128 : (m + 1) * 128])
                        )
                        nc.tensor.matmul(
                            pz[:, 0:hn],
                            r(b1m[:]),
                            r(ones_S[:, 0:hn]),
                            start=False,
                            stop=True,
                        )
                        et = scr.tile([128, 392], f32, tag="erf")
                        g_ = gp.tile([128, 392], f32, tag=f"g{m}")
                        nc.scalar.activation(
                            et[:, 0:hn], pz[:, 0:hn], AF.Erf, scale=INV_SQRT2
                        )
                        nc.vector.scalar_tensor_tensor(
                            r(g_[:, 0:hn]), et[:, 0:hn], 1.0, pz[:, 0:hn], OP.add, OP.mult
                        )
                        g.append(g_)
                    for m in range(DT):
                        pz2 = pbig.tile([128, S], f32, tag="pb", name="pb")
                        w2c = None
                        for kf in range(FT):
                            if kf % 6 == 0:
                                w2c = w2p.tile([128, 768], f32, tag="w2")
                                co = (kf // 6) * 768
                                nc.sync.dma_start(r(w2c), r(w2[lay, m, :, co : co + 768]))
                            off = (kf % 6) * 128
                            nc.tensor.matmul(
                                pz2[:, 0:hn],
                                r(w2c[:, off : off + 128]),
                                r(g[kf][:, 0:hn]),
                                start=(kf == 0),
                                stop=(kf == FT - 1),
                            )
                        nc.vector.scalar_tensor_tensor(
                            r(h[m][:, ho : ho + hn]),
                            pz2[:, 0:hn],
                            b2t[:, m : m + 1],
                            h[m][:, ho : ho + hn],
                            OP.add,
                            OP.add,
                        )

            # ---- final LN + store ----
            yf = [acts.tile([128, S], f32, tag=f"y{d}", name=f"yf{d}") for d in range(DT)]
            emit_ln(h, yf, final=True)
            for d in range(DT):
                nc.sync.dma_start(outT[d * 128 : (d + 1) * 128, :], yf[d][:])

    nc.finalize()
    return nc


def _prep(inputs, n_layers=L):
    """Host-side folds and layout rearrangement. Returns dict of dram arrays."""
    f = np.float32
    x = np.asarray(inputs["x"], f)
    ln1_s = np.asarray(inputs["ln1_s"], f)[:n_layers]
    ln1_b = np.asarray(inputs["ln1_b"], f)[:n_layers]
    wq = np.asarray(inputs["wq"], f)[:n_layers]
    bqv = np.asarray(inputs["bq"], f)[:n_layers]
    wk = np.asarray(inputs["wk"], f)[:n_layers]
    bkv = np.asarray(inputs["bk"], f)[:n_layers]
    wv = np.asarray(inputs["wv"], f)[:n_layers]
    bvv = np.asarray(inputs["bv"], f)[:n_layers]
    wo = np.asarray(inputs["wo"], f)[:n_layers]
    bov = np.asarray(inputs["bo"], f)[:n_layers]
    ln2_s = np.asarray(inputs["ln2_s"], f)[:n_layers]
    ln2_b = np.asarray(inputs["ln2_b"], f)[:n_layers]
    w1 = np.asarray(inputs["w1"], f)[:n_layers]
    b1v = np.asarray(inputs["b1"], f)[:n_layers]
    w2 = np.asarray(inputs["w2"], f)[:n_layers]
    b2v = np.asarray(inputs["b2"], f)[:n_layers]
    lnf_s = np.asarray(inputs["lnf_s"], f)
    lnf_b = np.asarray(inputs["lnf_b"], f)

    sc = 1.0 / np.sqrt(HD)
    Wq = ln1_s[:, :, None] * wq * sc
    bq_f = (np.einsum("ld,ldm->lm", ln1_b, wq) + bqv) * sc
    Wk = ln1_s[:, :, None] * wk
    bk_f = np.einsum("ld,ldm->lm", ln1_b, wk) + bkv
    Wv = ln1_s[:, :, None] * wv
    bv_f = np.einsum("ld,ldm->lm", ln1_b, wv) + bvv
    bo_f = bov + np.einsum("lm,lmn->ln", bv_f, wo)
    W1 = ln2_s[:, :, None] * w1
    b1_f = np.einsum("ld,ldf->lf", ln2_b, w1) + b1v
    W2h = w2 * 0.5

    def mtile(w, mt):
        lw = w.shape[0]
        kt = w.shape[1] // 128
        return np.ascontiguousarray(
            w.reshape(lw, kt, 128, mt, 128)
            .transpose(0, 3, 2, 1, 4)
            .reshape(lw, mt, 128, kt * 128)
        )

    def bcol(b):
        lw = b.shape[0]
        return np.ascontiguousarray(b.reshape(lw, DT, 128).transpose(0, 2, 1))

    arrs = {
        "wq": mtile(Wq, DT),
        "wk": mtile(Wk, DT),
        "wv": np.ascontiguousarray(Wv),
        "wo": mtile(wo, DT),
        "w1": mtile(W1, FT),
        "w2": mtile(W2h, DT),
        "bq": bcol(bq_f),
        "bk": bcol(bk_f),
        "bo": bcol(bo_f),
        "b2": bcol(b2v),
        "b1r": np.ascontiguousarray(b1_f),
        "lnf": np.ascontiguousarray(
            np.stack([lnf_s.reshape(DT, 128).T, lnf_b.reshape(DT, 128).T], -1)
        ),
        "onesd": np.ones((128, S), f),
    }
    xT = np.ascontiguousarray(x.transpose(0, 2, 1))  # [B, D, S]
    return arrs, xT


def kernel(**inputs):
    from concourse.bass_utils import run_bass_kernel_spmd

    n_layers = L
    if "nc" not in _CACHE:
        _CACHE["nc"] = _build(n_layers)
    nc = _CACHE["nc"]
    arrs, xT = _prep(inputs, n_layers)
    in_maps = [dict(arrs, xT=xT[c]) for c in range(B)]
    res = run_bass_kernel_spmd(nc, in_maps, core_ids=list(range(B)))
    out = np.stack([res.results[c]["outT"].T for c in range(B)])
    return np.ascontiguousarray(out.astype(np.float32))



# revision 7
# speedup vs baseline: 1.5896x; 1.5896x over previous
"""Trainium2 Bass kernel: 12-layer transformer encoder forward pass.

Strategy: pure data-parallel over batch — 8 NeuronCores x 1 batch element.
No collectives. Per core, activations are kept feature-on-partition
("f-layout", i.e. transposed: [D, S]) so every matmul consumes weights
as stored with zero on-chip transposes:

  out^T[M=dout, N=tok] = lhsT[K=din, M=dout].T @ rhs[K=din, N=tok]

All large matmuls run in bfloat16 (weights cast host-side; activations
quantized by the PSUM->SBUF mover ops). bf16 runs the PE at full rate
AND enables fast-weight-load (FWL is disabled for fp32), which the
baseline profile showed dominating (LDWEIGHTS ~254ns x 16k, PE stuck in
half-array mode). fp32 residual stream; fp32r only for the ones-matmul
LayerNorm statistics and row broadcasts.

LayerNorm rsqrt uses exp(-0.5*ln(var*D^2) + ln(D)) so the whole layer
needs only the natural_log_exp and gelu ACT table sets (2 loads/layer,
both hidden under PE work). GELU runs directly on the ACT engine with
the per-partition b1 bias fused. Q/K biases ride the DVE PSUM->SBUF
mover (tensor_scalar_add). Softmax: no max-subtraction (scores O(+-10));
sum(exp) via an appended ones-column on V; normalization via a
broadcast + fast reciprocal + one DVE multiply per head.
"""

import sys

sys.path.insert(0, "/opt/trn_rl_repo")

import math

import numpy as np

L = 12
D = 768
H = 12
F = 3072
B = 8
S = 784
HD = 64
DT = D // 128  # 6 feature tiles
FT = F // 128  # 24 mlp tiles
TT = (S + 127) // 128  # 7 token tiles, last has 16 rows
SPL = ((0, 512), (512, 272))  # token splits (bank-aligned psum)
VSPL = ((0, 512), (512, 256))  # dout splits for the v projection
LOG_D = math.log(float(D))

_CACHE = {}


def _tok(t):
    """(offset, rows) of token tile t."""
    return t * 128, (128 if t < TT - 1 else S - 128 * (TT - 1))


def _build(n_layers=L):
    import concourse.mybir as mybir
    from concourse import bacc
    from concourse.tile import TileContext

    f32 = mybir.dt.float32
    f32r = mybir.dt.float32r
    bf16 = mybir.dt.bfloat16
    AF = mybir.ActivationFunctionType
    OP = mybir.AluOpType

    def r(ap):
        return ap.bitcast(f32r)

    nc = bacc.Bacc("TRN2", target_bir_lowering=False)

    xT = nc.dram_tensor("xT", [D, S], f32, kind="ExternalInput")
    wq = nc.dram_tensor("wq", [n_layers, DT, 128, D], bf16, kind="ExternalInput")
    wk = nc.dram_tensor("wk", [n_layers, DT, 128, D], bf16, kind="ExternalInput")
    wv = nc.dram_tensor("wv", [n_layers, 128, DT * D], bf16, kind="ExternalInput")
    wo = nc.dram_tensor("wo", [n_layers, DT, 128, D], bf16, kind="ExternalInput")
    w1 = nc.dram_tensor("w1", [n_layers, FT, 128, D], bf16, kind="ExternalInput")
    w2 = nc.dram_tensor("w2", [n_layers, DT, 128, F], bf16, kind="ExternalInput")
    # bias columns: [bq(6) bk(6) bo(6) b2(6) b1(24)] = 48
    bias = nc.dram_tensor("bias", [n_layers, 128, 48], f32, kind="ExternalInput")
    lnf = nc.dram_tensor("lnf", [128, DT, 2], f32, kind="ExternalInput")
    onesd = nc.dram_tensor("onesd", [128, S], f32, kind="ExternalInput")
    outT = nc.dram_tensor("outT", [D, S], f32, kind="ExternalOutput")

    with TileContext(nc) as tc:
        from contextlib import ExitStack

        with ExitStack() as ctx:
            perm = ctx.enter_context(tc.tile_pool(name="perm", bufs=1))
            qkp = ctx.enter_context(tc.tile_pool(name="qk", bufs=1))
            cxp = ctx.enter_context(tc.tile_pool(name="cxp", bufs=1))
            acts = ctx.enter_context(tc.tile_pool(name="acts", bufs=1))
            gp = ctx.enter_context(tc.tile_pool(name="gp", bufs=1))
            sqp = ctx.enter_context(tc.tile_pool(name="sqp", bufs=3))
            scrf = ctx.enter_context(tc.tile_pool(name="scrf", bufs=2))
            ep = ctx.enter_context(tc.tile_pool(name="ep", bufs=3))
            rbp = ctx.enter_context(tc.tile_pool(name="rbp", bufs=2))
            wqp = ctx.enter_context(tc.tile_pool(name="wqp", bufs=3))
            wvp = ctx.enter_context(tc.tile_pool(name="wvp", bufs=2))
            w1p = ctx.enter_context(tc.tile_pool(name="w1p", bufs=3))
            w2p = ctx.enter_context(tc.tile_pool(name="w2p", bufs=2))
            bp = ctx.enter_context(tc.tile_pool(name="bp", bufs=2))
            pA = ctx.enter_context(tc.tile_pool(name="pA", bufs=2, space="PSUM"))
            pB = ctx.enter_context(tc.tile_pool(name="pB", bufs=2, space="PSUM"))

            # persistent tiles
            h = [perm.tile([128, S], f32, tag=f"h{d}", name=f"h{d}") for d in range(DT)]
            v = [perm.tile([128, H * 65], bf16, tag=f"v{t}", name=f"v{t}") for t in range(TT)]
            ones = perm.tile([128, S], f32, tag="ones")
            stat = perm.tile([128, S], f32, tag="stat")
            statb = perm.tile([128, S], f32, tag="statb")
            statc = perm.tile([128, S], f32, tag="statc")
            statd = perm.tile([128, S], f32, tag="statd")
            lnft = perm.tile([128, DT * 2], f32, tag="lnft")
            logd = perm.tile([128, 1], f32, tag="logd")
            nc.vector.memset(logd[:], LOG_D)
            cx = [cxp.tile([128, S], bf16, tag=f"c{d}", name=f"c{d}") for d in range(DT)]

            nc.sync.dma_start(r(ones), r(onesd[:, :]))
            nc.sync.dma_start(lnft[:], lnf[:, :, :])
            for d in range(DT):
                nc.sync.dma_start(r(h[d]), r(xT[d * 128 : (d + 1) * 128, :]))
            # ones columns of the augmented V (column 64 of each head's 65)
            for t in range(TT):
                vap = v[t][:].rearrange("p (a c) -> p a c", c=65)[:, :, 64:65]
                nc.vector.memset(vap, 1.0)

            ones_col = ones[:, 0:1]  # lhsT [K=128, M=1] for partition sums

            def emit_ln(src, dst, final=False):
                """dst[d] = (src[d] - mean)*rsqrt(var) feature-on-partition.
                dst tiles may be bf16. If final, apply lnf scale/bias."""
                ps_s = pA.tile([128, S], f32, tag="pa", name="lnps")
                ps_q = pA.tile([128, S], f32, tag="pa", name="lnpq")
                for d in range(DT):
                    sqt = sqp.tile([128, S], f32, tag="sq")
                    nc.vector.tensor_tensor(r(sqt), src[d][:], src[d][:], OP.mult)
                    for o, n in SPL:
                        nc.tensor.matmul(
                            ps_s[0:1, o : o + n], r(ones_col), r(src[d][:, o : o + n]),
                            start=(d == 0), stop=(d == DT - 1),
                        )
                    for o, n in SPL:
                        nc.tensor.matmul(
                            ps_q[0:1, o : o + n], r(ones_col), r(sqt[:, o : o + n]),
                            start=(d == 0), stop=(d == DT - 1),
                        )
                # rows: var*D^2 = q*D - s^2 ; a = 1/sqrt(var) = exp(-.5*ln(t)+ln(D))
                # c = -mean*a. Bases follow the baseline's proven mixed-base ops.
                sq_row = statd[96:97, :]
                t_row = stat[0:1, :]
                lt_row = statd[0:1, :]
                a_row = statb[64:65, :]
                c_row = statb[32:33, :]
                nc.scalar.activation(sq_row, ps_s[0:1, :], AF.Square)
                nc.vector.scalar_tensor_tensor(
                    t_row, ps_q[0:1, :], float(D), sq_row, OP.mult, OP.subtract
                )
                nc.scalar.activation(lt_row, t_row, AF.Ln)
                nc.scalar.activation(
                    r(a_row), lt_row, AF.Exp, scale=-0.5, bias=logd[0:1, 0:1]
                )
                nc.vector.scalar_tensor_tensor(
                    r(c_row), ps_s[0:1, :], -1.0 / D, a_row, OP.mult, OP.mult
                )
                ps_a = pB.tile([128, S], f32, tag="pb", name="bca")
                ps_c = pB.tile([128, S], f32, tag="pb", name="bcc")
                for o, n in SPL:
                    nc.tensor.matmul(
                        ps_a[:, o : o + n], r(ones[64:65, 0:128]), r(a_row[:, o : o + n]),
                        start=True, stop=True,
                    )
                    nc.tensor.matmul(
                        ps_c[:, o : o + n], r(ones[32:33, 0:128]), r(c_row[:, o : o + n]),
                        start=True, stop=True,
                    )
                for d in range(DT):
                    tmp = scrf.tile([128, S], f32, tag="lntmp")
                    nc.vector.tensor_tensor(tmp[:], src[d][:], ps_a[:, :], OP.mult)
                    nc.vector.tensor_tensor(dst[d][:], tmp[:], ps_c[:, :], OP.add)
                    if final:
                        nc.vector.tensor_scalar(
                            out=dst[d][:],
                            in0=dst[d][:],
                            scalar1=lnft[:, 2 * d : 2 * d + 1],
                            scalar2=lnft[:, 2 * d + 1 : 2 * d + 2],
                            op0=OP.mult,
                            op1=OP.add,
                        )

            for lay in range(n_layers):
                # ---- biases for this layer ----
                bt = bp.tile([128, 48], f32, tag="bias")
                nc.sync.dma_start(bt[:], bias[lay])

                # ---- LN1 ----
                y = [acts.tile([128, S], bf16, tag=f"y{d}", name=f"y1l{d}") for d in range(DT)]
                emit_ln(h, y)

                # ---- V projection (token-on-partition, into augmented V) ----
                wvt = wvp.tile([128, DT * D], bf16, tag="wv")
                nc.sync.dma_start(wvt[:], wv[lay])
                for o, n in VSPL:
                    a0 = o // 64
                    na = n // 64
                    for t in range(TT):
                        to, tm = _tok(t)
                        pv = pA.tile([128, S], f32, tag="pa", name="pv")
                        for kd in range(DT):
                            nc.tensor.matmul(
                                pv[0:tm, 0:n],
                                y[kd][:, to : to + tm],
                                wvt[:, kd * D + o : kd * D + o + n],
                                start=(kd == 0),
                                stop=(kd == DT - 1),
                            )
                        dst = v[t][:].rearrange("p (a c) -> p a c", c=65)[
                            0:tm, a0 : a0 + na, 0:64
                        ]
                        srcv = pv[0:tm, 0:n].rearrange("p (a c) -> p a c", c=64)
                        nc.vector.tensor_copy(dst, srcv)

                # ---- Q/K projections (all heads up front) ----
                q6 = [qkp.tile([128, S], bf16, tag=f"q{d}", name=f"q{d}") for d in range(DT)]
                k6 = [qkp.tile([128, S], bf16, tag=f"k{d}", name=f"k{d}") for d in range(DT)]
                for m in range(DT):
                    for dst6, wsrc, boff in ((q6, wq, 0), (k6, wk, 6)):
                        wt = wqp.tile([128, D], bf16, tag="wa")
                        nc.sync.dma_start(wt[:], wsrc[lay, m])
                        pq = pA.tile([128, S], f32, tag="pa", name="pq")
                        for o, n in SPL:
                            for kd in range(DT):
                                nc.tensor.matmul(
                                    pq[:, o : o + n],
                                    wt[:, kd * 128 : (kd + 1) * 128],
                                    y[kd][:, o : o + n],
                                    start=(kd == 0),
                                    stop=(kd == DT - 1),
                                )
                        nc.vector.tensor_scalar_add(
                            dst6[m][:], pq[:, :], bt[:, boff + m : boff + m + 1]
                        )

                # ---- attention heads ----
                for hd_ in range(H):
                    j, po = hd_ // 2, (hd_ % 2) * 64
                    psC = pB.tile([128, S], f32, tag="pb", name=f"psC{hd_}")
                    for t in range(TT):
                        to, tm = _tok(t)
                        ps = pA.tile([128, S], f32, tag="pa", name="ps")
                        for o, n in SPL:
                            nc.tensor.matmul(
                                ps[0:tm, o : o + n],
                                k6[j][po : po + 64, to : to + tm],
                                q6[j][po : po + 64, o : o + n],
                                start=True,
                                stop=True,
                            )
                        e = ep.tile([128, S], bf16, tag="e")
                        nc.scalar.activation(e[0:tm, :], ps[0:tm, :], AF.Exp)
                        for o, n in SPL:
                            nc.tensor.matmul(
                                psC[0:65, o : o + n],
                                v[t][0:tm, hd_ * 65 : hd_ * 65 + 65],
                                e[0:tm, o : o + n],
                                start=(t == 0),
                                stop=(t == TT - 1),
                            )
                    # softmax tail: broadcast sum(exp) (row 64), reciprocal,
                    # then scale raw context into cx.
                    rp = (hd_ % 2) * 32
                    serow = statc[rp : rp + 1, :]
                    nc.vector.tensor_copy(r(serow), psC[64:65, :])
                    prr = pA.tile([128, S], f32, tag="pa", name="prr")
                    for o, n in SPL:
                        nc.tensor.matmul(
                            prr[:, o : o + n],
                            r(ones[rp : rp + 1, 0:128]),
                            r(serow[:, o : o + n]),
                            start=True,
                            stop=True,
                        )
                    rb = rbp.tile([128, S], f32, tag="rb")
                    nc.vector.reciprocal_approx_fast(out=rb[:, :], in_=prr[:, :])
                    nc.vector.tensor_tensor(
                        cx[j][po : po + 64, :],
                        psC[0:64, :],
                        rb[po : po + 64, :],
                        OP.mult,
                    )

                # ---- O projection + residual ----
                for m in range(DT):
                    wt = wqp.tile([128, D], bf16, tag="wa")
                    nc.sync.dma_start(wt[:], wo[lay, m])
                    po_ = pA.tile([128, S], f32, tag="pa", name="po")
                    for o, n in SPL:
                        for kd in range(DT):
                            nc.tensor.matmul(
                                po_[:, o : o + n],
                                wt[:, kd * 128 : (kd + 1) * 128],
                                cx[kd][:, o : o + n],
                                start=(kd == 0),
                                stop=(kd == DT - 1),
                            )
                    nc.vector.scalar_tensor_tensor(
                        r(h[m][:]), po_[:, :], bt[:, 12 + m : 13 + m], h[m][:], OP.add, OP.add
                    )

                # ---- LN2 ----
                y2 = [acts.tile([128, S], bf16, tag=f"y{d}", name=f"y2l{d}") for d in range(DT)]
                emit_ln(h, y2)

                # ---- MLP fc1: full-width tiles, GELU with fused bias ----
                g = []
                for m in range(FT):
                    w1t = w1p.tile([128, D], bf16, tag="w1")
                    nc.sync.dma_start(w1t[:], w1[lay, m])
                    pz = pA.tile([128, S], f32, tag="pa", name="pz")
                    for o, n in SPL:
                        for kd in range(DT):
                            nc.tensor.matmul(
                                pz[:, o : o + n],
                                w1t[:, kd * 128 : (kd + 1) * 128],
                                y2[kd][:, o : o + n],
                                start=(kd == 0),
                                stop=(kd == DT - 1),
                            )
                    g_ = gp.tile([128, S], bf16, tag=f"g{m}")
                    nc.scalar.activation(
                        g_[:, :], pz[:, :], AF.Gelu, bias=bt[:, 24 + m : 25 + m]
                    )
                    g.append(g_)

                # ---- MLP fc2 + residual ----
                for m in range(DT):
                    w2t = w2p.tile([128, F], bf16, tag="w2")
                    nc.sync.dma_start(w2t[:], w2[lay, m])
                    pz2 = pB.tile([128, S], f32, tag="pb", name="pz2")
                    for o, n in SPL:
                        for kf in range(FT):
                            nc.tensor.matmul(
                                pz2[:, o : o + n],
                                w2t[:, kf * 128 : (kf + 1) * 128],
                                g[kf][:, o : o + n],
                                start=(kf == 0),
                                stop=(kf == FT - 1),
                            )
                    nc.vector.scalar_tensor_tensor(
                        r(h[m][:]), pz2[:, :], bt[:, 18 + m : 19 + m], h[m][:], OP.add, OP.add
                    )

            # ---- final LN + store ----
            yf = [acts.tile([128, S], f32, tag=f"y{d}", name=f"yf{d}") for d in range(DT)]
            emit_ln(h, yf, final=True)
            for d in range(DT):
                nc.sync.dma_start(outT[d * 128 : (d + 1) * 128, :], yf[d][:])

    nc.finalize()
    return nc


def _prep(inputs, n_layers=L):
    """Host-side folds and layout rearrangement. Returns dict of dram arrays."""
    import ml_dtypes

    f = np.float32
    bf = ml_dtypes.bfloat16
    x = np.asarray(inputs["x"], f)
    ln1_s = np.asarray(inputs["ln1_s"], f)[:n_layers]
    ln1_b = np.asarray(inputs["ln1_b"], f)[:n_layers]
    wq = np.asarray(inputs["wq"], f)[:n_layers]
    bqv = np.asarray(inputs["bq"], f)[:n_layers]
    wk = np.asarray(inputs["wk"], f)[:n_layers]
    bkv = np.asarray(inputs["bk"], f)[:n_layers]
    wv = np.asarray(inputs["wv"], f)[:n_layers]
    bvv = np.asarray(inputs["bv"], f)[:n_layers]
    wo = np.asarray(inputs["wo"], f)[:n_layers]
    bov = np.asarray(inputs["bo"], f)[:n_layers]
    ln2_s = np.asarray(inputs["ln2_s"], f)[:n_layers]
    ln2_b = np.asarray(inputs["ln2_b"], f)[:n_layers]
    w1 = np.asarray(inputs["w1"], f)[:n_layers]
    b1v = np.asarray(inputs["b1"], f)[:n_layers]
    w2 = np.asarray(inputs["w2"], f)[:n_layers]
    b2v = np.asarray(inputs["b2"], f)[:n_layers]
    lnf_s = np.asarray(inputs["lnf_s"], f)
    lnf_b = np.asarray(inputs["lnf_b"], f)

    sc = 1.0 / np.sqrt(HD)
    Wq = ln1_s[:, :, None] * wq * sc
    bq_f = (np.einsum("ld,ldm->lm", ln1_b, wq) + bqv) * sc
    Wk = ln1_s[:, :, None] * wk
    bk_f = np.einsum("ld,ldm->lm", ln1_b, wk) + bkv
    Wv = ln1_s[:, :, None] * wv
    bv_f = np.einsum("ld,ldm->lm", ln1_b, wv) + bvv
    bo_f = bov + np.einsum("lm,lmn->ln", bv_f, wo)
    W1 = ln2_s[:, :, None] * w1
    b1_f = np.einsum("ld,ldf->lf", ln2_b, w1) + b1v

    lw = n_layers

    def mtile(w, mt):
        kt = w.shape[1] // 128
        return np.ascontiguousarray(
            w.reshape(lw, kt, 128, mt, 128)
            .transpose(0, 3, 2, 1, 4)
            .reshape(lw, mt, 128, kt * 128)
        ).astype(bf)

    def bcol(b):
        return b.reshape(lw, DT, 128).transpose(0, 2, 1)

    bias = np.concatenate(
        [
            bcol(bq_f),
            bcol(bk_f),
            bcol(bo_f),
            bcol(b2v),
            b1_f.reshape(lw, FT, 128).transpose(0, 2, 1),
        ],
        axis=2,
    )

    arrs = {
        "wq": mtile(Wq, DT),
        "wk": mtile(Wk, DT),
        # wv packed kd-major along columns: [128, kd*D + c]
        "wv": np.ascontiguousarray(
            Wv.reshape(lw, DT, 128, D).transpose(0, 2, 1, 3).reshape(lw, 128, DT * D)
        ).astype(bf),
        "wo": mtile(wo, DT),
        "w1": mtile(W1, FT),
        "w2": mtile(w2, DT),
        "bias": np.ascontiguousarray(bias.astype(f)),
        "lnf": np.ascontiguousarray(
            np.stack([lnf_s.reshape(DT, 128).T, lnf_b.reshape(DT, 128).T], -1)
        ),
        "onesd": np.ones((128, S), f),
    }
    xT = np.ascontiguousarray(x.transpose(0, 2, 1))  # [B, D, S]
    return arrs, xT


def kernel(**inputs):
    from concourse.bass_utils import run_bass_kernel_spmd

    n_layers = L
    if "nc" not in _CACHE:
        _CACHE["nc"] = _build(n_layers)
    nc = _CACHE["nc"]
    arrs, xT = _prep(inputs, n_layers)
    in_maps = [dict(arrs, xT=xT[c]) for c in range(B)]
    res = run_bass_kernel_spmd(nc, in_maps, core_ids=list(range(B)))
    out = np.stack([res.results[c]["outT"].T for c in range(B)])
    return np.ascontiguousarray(out.astype(np.float32))


# revision 8
# speedup vs baseline: 1.6117x; 1.0139x over previous
"""Trainium2 Bass kernel: 12-layer transformer encoder forward pass.

Strategy: pure data-parallel over batch — 8 NeuronCores x 1 batch element.
No collectives. Per core, activations are kept feature-on-partition
("f-layout", i.e. transposed: [D, S]) so every matmul consumes weights
as stored with zero on-chip transposes:

  out^T[M=dout, N=tok] = lhsT[K=din, M=dout].T @ rhs[K=din, N=tok]

All large matmuls run in bfloat16 (weights cast host-side; activations
quantized by the PSUM->SBUF mover ops). bf16 runs the PE at full rate
AND enables fast-weight-load (FWL is disabled for fp32), which the
baseline profile showed dominating (LDWEIGHTS ~254ns x 16k, PE stuck in
half-array mode). fp32 residual stream; fp32r only for the ones-matmul
LayerNorm statistics and row broadcasts.

LayerNorm rsqrt uses exp(-0.5*ln(var*D^2) + ln(D)) so the whole layer
needs only the natural_log_exp and gelu ACT table sets (2 loads/layer,
both hidden under PE work). GELU runs directly on the ACT engine with
the per-partition b1 bias fused. Q/K biases ride the DVE PSUM->SBUF
mover (tensor_scalar_add). Softmax: no max-subtraction (scores O(+-10));
sum(exp) via an appended ones-column on V; normalization via a
broadcast + fast reciprocal + one DVE multiply per head.
"""

import sys

sys.path.insert(0, "/opt/trn_rl_repo")

import math

import numpy as np

L = 12
D = 768
H = 12
F = 3072
B = 8
S = 784
HD = 64
DT = D // 128  # 6 feature tiles
FT = F // 128  # 24 mlp tiles
TT = (S + 127) // 128  # 7 token tiles, last has 16 rows
SPL = ((0, 512), (512, 272))  # token splits (bank-aligned psum)
VSPL = ((0, 512), (512, 256))  # dout splits for the v projection
LOG_D = math.log(float(D))

_CACHE = {}


def _tok(t):
    """(offset, rows) of token tile t."""
    return t * 128, (128 if t < TT - 1 else S - 128 * (TT - 1))


def _build(n_layers=L):
    import concourse.mybir as mybir
    from concourse import bacc
    from concourse.tile import TileContext

    f32 = mybir.dt.float32
    f32r = mybir.dt.float32r
    bf16 = mybir.dt.bfloat16
    AF = mybir.ActivationFunctionType
    OP = mybir.AluOpType

    def r(ap):
        return ap.bitcast(f32r)

    nc = bacc.Bacc("TRN2", target_bir_lowering=False)

    xT = nc.dram_tensor("xT", [D, S], f32, kind="ExternalInput")
    wq = nc.dram_tensor("wq", [n_layers, DT, 128, D], bf16, kind="ExternalInput")
    wk = nc.dram_tensor("wk", [n_layers, DT, 128, D], bf16, kind="ExternalInput")
    wv = nc.dram_tensor("wv", [n_layers, 128, DT * D], bf16, kind="ExternalInput")
    wo = nc.dram_tensor("wo", [n_layers, DT, 128, D], bf16, kind="ExternalInput")
    w1 = nc.dram_tensor("w1", [n_layers, FT, 128, D], bf16, kind="ExternalInput")
    w2 = nc.dram_tensor("w2", [n_layers, DT, 128, F], bf16, kind="ExternalInput")
    # bias columns: [bq(6) bk(6) bo(6) b2(6) b1(24)] = 48
    bias = nc.dram_tensor("bias", [n_layers, 128, 48], f32, kind="ExternalInput")
    lnf = nc.dram_tensor("lnf", [128, DT, 2], f32, kind="ExternalInput")
    onesd = nc.dram_tensor("onesd", [128, S], f32, kind="ExternalInput")
    outT = nc.dram_tensor("outT", [D, S], f32, kind="ExternalOutput")

    with TileContext(nc) as tc:
        from contextlib import ExitStack

        with ExitStack() as ctx:
            perm = ctx.enter_context(tc.tile_pool(name="perm", bufs=1))
            qkp = ctx.enter_context(tc.tile_pool(name="qk", bufs=1))
            cxp = ctx.enter_context(tc.tile_pool(name="cxp", bufs=1))
            acts = ctx.enter_context(tc.tile_pool(name="acts", bufs=1))
            gp = ctx.enter_context(tc.tile_pool(name="gp", bufs=1))
            sqp = ctx.enter_context(tc.tile_pool(name="sqp", bufs=3))
            scrf = ctx.enter_context(tc.tile_pool(name="scrf", bufs=2))
            ep = ctx.enter_context(tc.tile_pool(name="ep", bufs=3))
            rbp = ctx.enter_context(tc.tile_pool(name="rbp", bufs=2))
            wqp = ctx.enter_context(tc.tile_pool(name="wqp", bufs=3))
            wvp = ctx.enter_context(tc.tile_pool(name="wvp", bufs=2))
            w1p = ctx.enter_context(tc.tile_pool(name="w1p", bufs=3))
            w2p = ctx.enter_context(tc.tile_pool(name="w2p", bufs=2))
            bp = ctx.enter_context(tc.tile_pool(name="bp", bufs=2))
            pA = ctx.enter_context(tc.tile_pool(name="pA", bufs=2, space="PSUM"))
            pB = ctx.enter_context(tc.tile_pool(name="pB", bufs=2, space="PSUM"))

            # persistent tiles
            h = [perm.tile([128, S], f32, tag=f"h{d}", name=f"h{d}") for d in range(DT)]
            v = [perm.tile([128, H * 65], bf16, tag=f"v{t}", name=f"v{t}") for t in range(TT)]
            ones = perm.tile([128, S], f32, tag="ones")
            stat = perm.tile([128, S], f32, tag="stat")
            statb = perm.tile([128, S], f32, tag="statb")
            statc = perm.tile([128, S], f32, tag="statc")
            statd = perm.tile([128, S], f32, tag="statd")
            lnft = perm.tile([128, DT * 2], f32, tag="lnft")
            logd = perm.tile([128, 1], f32, tag="logd")
            nc.vector.memset(logd[:], LOG_D)
            cx = [cxp.tile([128, S], bf16, tag=f"c{d}", name=f"c{d}") for d in range(DT)]

            nc.sync.dma_start(r(ones), r(onesd[:, :]))
            nc.sync.dma_start(lnft[:], lnf[:, :, :])
            for d in range(DT):
                nc.sync.dma_start(r(h[d]), r(xT[d * 128 : (d + 1) * 128, :]))
            # ones columns of the augmented V (column 64 of each head's 65)
            for t in range(TT):
                vap = v[t][:].rearrange("p (a c) -> p a c", c=65)[:, :, 64:65]
                nc.vector.memset(vap, 1.0)

            ones_col = ones[:, 0:1]  # lhsT [K=128, M=1] for partition sums

            def emit_ln(src, dst, final=False):
                """dst[d] = (src[d] - mean)*rsqrt(var) feature-on-partition.
                dst tiles may be bf16. If final, apply lnf scale/bias."""
                ps_s = pA.tile([128, S], f32, tag="pa", name="lnps")
                ps_q = pA.tile([128, S], f32, tag="pa", name="lnpq")
                for d in range(DT):
                    sqt = sqp.tile([128, S], f32, tag="sq")
                    nc.vector.tensor_tensor(r(sqt), src[d][:], src[d][:], OP.mult)
                    for o, n in SPL:
                        nc.tensor.matmul(
                            ps_s[0:1, o : o + n], r(ones_col), r(src[d][:, o : o + n]),
                            start=(d == 0), stop=(d == DT - 1),
                        )
                    for o, n in SPL:
                        nc.tensor.matmul(
                            ps_q[0:1, o : o + n], r(ones_col), r(sqt[:, o : o + n]),
                            start=(d == 0), stop=(d == DT - 1),
                        )
                # rows: var*D^2 = q*D - s^2 ; a = 1/sqrt(var) = exp(-.5*ln(t)+ln(D))
                # c = -mean*a. Bases follow the baseline's proven mixed-base ops.
                sq_row = statd[96:97, :]
                t_row = stat[0:1, :]
                lt_row = statd[0:1, :]
                a_row = statb[64:65, :]
                c_row = statb[32:33, :]
                nc.scalar.activation(sq_row, ps_s[0:1, :], AF.Square)
                nc.vector.scalar_tensor_tensor(
                    t_row, ps_q[0:1, :], float(D), sq_row, OP.mult, OP.subtract
                )
                nc.scalar.activation(lt_row, t_row, AF.Ln)
                nc.scalar.activation(
                    r(a_row), lt_row, AF.Exp, scale=-0.5, bias=logd[0:1, 0:1]
                )
                nc.vector.scalar_tensor_tensor(
                    r(c_row), ps_s[0:1, :], -1.0 / D, a_row, OP.mult, OP.mult
                )
                ps_a = pB.tile([128, S], f32, tag="pb", name="bca")
                ps_c = pB.tile([128, S], f32, tag="pb", name="bcc")
                for o, n in SPL:
                    nc.tensor.matmul(
                        ps_a[:, o : o + n], r(ones[64:65, 0:128]), r(a_row[:, o : o + n]),
                        start=True, stop=True,
                    )
                    nc.tensor.matmul(
                        ps_c[:, o : o + n], r(ones[32:33, 0:128]), r(c_row[:, o : o + n]),
                        start=True, stop=True,
                    )
                for d in range(DT):
                    tmp = scrf.tile([128, S], f32, tag="lntmp")
                    nc.vector.tensor_tensor(tmp[:], src[d][:], ps_a[:, :], OP.mult)
                    nc.vector.tensor_tensor(dst[d][:], tmp[:], ps_c[:, :], OP.add)
                    if final:
                        nc.vector.tensor_scalar(
                            out=dst[d][:],
                            in0=dst[d][:],
                            scalar1=lnft[:, 2 * d : 2 * d + 1],
                            scalar2=lnft[:, 2 * d + 1 : 2 * d + 2],
                            op0=OP.mult,
                            op1=OP.add,
                        )

            for lay in range(n_layers):
                # ---- biases for this layer ----
                bt = bp.tile([128, 48], f32, tag="bias")
                nc.sync.dma_start(bt[:], bias[lay])

                # ---- LN1 ----
                y = [acts.tile([128, S], bf16, tag=f"y{d}", name=f"y1l{d}") for d in range(DT)]
                emit_ln(h, y)

                # ---- attention: V/Q/K projections interleaved with heads so
                # the PE stays dense (k=8 HAM) while ACT streams the exps ----
                wvt = wvp.tile([128, DT * D], bf16, tag="wv")
                nc.sync.dma_start(wvt[:], wv[lay])
                q6 = [qkp.tile([128, S], bf16, tag=f"q{d}", name=f"q{d}") for d in range(DT)]
                k6 = [qkp.tile([128, S], bf16, tag=f"k{d}", name=f"k{d}") for d in range(DT)]

                def emit_vproj(o, n):
                    a0 = o // 64
                    na = n // 64
                    for t in range(TT):
                        to, tm = _tok(t)
                        pv = pA.tile([128, S], f32, tag="pa", name="pv")
                        for kd in range(DT):
                            nc.tensor.matmul(
                                pv[0:tm, 0:n],
                                y[kd][:, to : to + tm],
                                wvt[:, kd * D + o : kd * D + o + n],
                                start=(kd == 0),
                                stop=(kd == DT - 1),
                            )
                        dst = v[t][:].rearrange("p (a c) -> p a c", c=65)[
                            0:tm, a0 : a0 + na, 0:64
                        ]
                        srcv = pv[0:tm, 0:n].rearrange("p (a c) -> p a c", c=64)
                        nc.vector.tensor_copy(dst, srcv)

                def emit_qk(m):
                    for dst6, wsrc, boff in ((q6, wq, 0), (k6, wk, 6)):
                        wt = wqp.tile([128, D], bf16, tag="wa")
                        nc.sync.dma_start(wt[:], wsrc[lay, m])
                        pq = pA.tile([128, S], f32, tag="pa", name="pq")
                        for o, n in SPL:
                            for kd in range(DT):
                                nc.tensor.matmul(
                                    pq[:, o : o + n],
                                    wt[:, kd * 128 : (kd + 1) * 128],
                                    y[kd][:, o : o + n],
                                    start=(kd == 0),
                                    stop=(kd == DT - 1),
                                )
                        nc.vector.tensor_scalar_add(
                            dst6[m][:], pq[:, :], bt[:, boff + m : boff + m + 1]
                        )

                wots = []

                def emit_head(hd_):
                    j, po = hd_ // 2, (hd_ % 2) * 64
                    psC = pB.tile([128, S], f32, tag="pb", name=f"psC{hd_}")
                    for t in range(TT):
                        to, tm = _tok(t)
                        ps = pA.tile([128, S], f32, tag="pa", name="ps")
                        for o, n in SPL:
                            nc.tensor.matmul(
                                ps[0:tm, o : o + n],
                                k6[j][po : po + 64, to : to + tm],
                                q6[j][po : po + 64, o : o + n],
                                start=True,
                                stop=True,
                            )
                        e = ep.tile([128, S], bf16, tag="e")
                        nc.scalar.activation(e[0:tm, :], ps[0:tm, :], AF.Exp)
                        for o, n in SPL:
                            nc.tensor.matmul(
                                psC[0:65, o : o + n],
                                v[t][0:tm, hd_ * 65 : hd_ * 65 + 65],
                                e[0:tm, o : o + n],
                                start=(t == 0),
                                stop=(t == TT - 1),
                            )
                    # softmax tail: broadcast sum(exp) (row 64), reciprocal,
                    # then scale raw context into cx.
                    rp = (hd_ % 2) * 32
                    serow = statc[rp : rp + 1, :]
                    nc.vector.tensor_copy(r(serow), psC[64:65, :])
                    prr = pA.tile([128, S], f32, tag="pa", name="prr")
                    for o, n in SPL:
                        nc.tensor.matmul(
                            prr[:, o : o + n],
                            r(ones[rp : rp + 1, 0:128]),
                            r(serow[:, o : o + n]),
                            start=True,
                            stop=True,
                        )
                    rb = rbp.tile([128, S], f32, tag="rb")
                    nc.vector.reciprocal_approx_fast(out=rb[:, :], in_=prr[:, :])
                    nc.vector.tensor_tensor(
                        cx[j][po : po + 64, :],
                        psC[0:64, :],
                        rb[po : po + 64, :],
                        OP.mult,
                    )

                emit_vproj(*VSPL[0])
                emit_qk(0)
                emit_head(0)
                emit_qk(1)
                emit_head(1)
                emit_qk(2)
                emit_head(2)
                emit_qk(3)
                emit_head(3)
                emit_vproj(*VSPL[1])
                emit_head(4)
                emit_qk(4)
                emit_head(5)
                emit_qk(5)
                for hd_ in range(6, H):
                    if hd_ < 9:  # prefetch O weights during the head stream
                        wt = wqp.tile([128, D], bf16, tag="wa")
                        nc.sync.dma_start(wt[:], wo[lay, hd_ - 6])
                        wots.append(wt)
                    emit_head(hd_)

                # ---- O projection + residual ----
                for m in range(DT):
                    if m < 3:
                        wt = wots[m]
                    else:
                        wt = wqp.tile([128, D], bf16, tag="wa")
                        nc.sync.dma_start(wt[:], wo[lay, m])
                    po_ = pA.tile([128, S], f32, tag="pa", name="po")
                    for o, n in SPL:
                        for kd in range(DT):
                            nc.tensor.matmul(
                                po_[:, o : o + n],
                                wt[:, kd * 128 : (kd + 1) * 128],
                                cx[kd][:, o : o + n],
                                start=(kd == 0),
                                stop=(kd == DT - 1),
                            )
                    nc.vector.scalar_tensor_tensor(
                        r(h[m][:]), po_[:, :], bt[:, 12 + m : 13 + m], h[m][:], OP.add, OP.add
                    )

                # ---- LN2 ----
                y2 = [acts.tile([128, S], bf16, tag=f"y{d}", name=f"y2l{d}") for d in range(DT)]
                emit_ln(h, y2)

                # ---- MLP fc1: full-width tiles, GELU with fused bias ----
                g = []
                for m in range(FT):
                    w1t = w1p.tile([128, D], bf16, tag="w1")
                    nc.sync.dma_start(w1t[:], w1[lay, m])
                    pz = pA.tile([128, S], f32, tag="pa", name="pz")
                    for o, n in SPL:
                        for kd in range(DT):
                            nc.tensor.matmul(
                                pz[:, o : o + n],
                                w1t[:, kd * 128 : (kd + 1) * 128],
                                y2[kd][:, o : o + n],
                                start=(kd == 0),
                                stop=(kd == DT - 1),
                            )
                    g_ = gp.tile([128, S], bf16, tag=f"g{m}")
                    nc.scalar.activation(
                        g_[:, :], pz[:, :], AF.Gelu, bias=bt[:, 24 + m : 25 + m]
                    )
                    g.append(g_)

                # ---- MLP fc2 + residual ----
                for m in range(DT):
                    w2t = w2p.tile([128, F], bf16, tag="w2")
                    nc.sync.dma_start(w2t[:], w2[lay, m])
                    pz2 = pB.tile([128, S], f32, tag="pb", name="pz2")
                    for o, n in SPL:
                        for kf in range(FT):
                            nc.tensor.matmul(
                                pz2[:, o : o + n],
                                w2t[:, kf * 128 : (kf + 1) * 128],
                                g[kf][:, o : o + n],
                                start=(kf == 0),
                                stop=(kf == FT - 1),
                            )
                    nc.vector.scalar_tensor_tensor(
                        r(h[m][:]), pz2[:, :], bt[:, 18 + m : 19 + m], h[m][:], OP.add, OP.add
                    )

            # ---- final LN + store ----
            yf = [acts.tile([128, S], f32, tag=f"y{d}", name=f"yf{d}") for d in range(DT)]
            emit_ln(h, yf, final=True)
            for d in range(DT):
                nc.sync.dma_start(outT[d * 128 : (d + 1) * 128, :], yf[d][:])

    nc.finalize()
    return nc


def _prep(inputs, n_layers=L):
    """Host-side folds and layout rearrangement. Returns dict of dram arrays."""
    import ml_dtypes

    f = np.float32
    bf = ml_dtypes.bfloat16
    x = np.asarray(inputs["x"], f)
    ln1_s = np.asarray(inputs["ln1_s"], f)[:n_layers]
    ln1_b = np.asarray(inputs["ln1_b"], f)[:n_layers]
    wq = np.asarray(inputs["wq"], f)[:n_layers]
    bqv = np.asarray(inputs["bq"], f)[:n_layers]
    wk = np.asarray(inputs["wk"], f)[:n_layers]
    bkv = np.asarray(inputs["bk"], f)[:n_layers]
    wv = np.asarray(inputs["wv"], f)[:n_layers]
    bvv = np.asarray(inputs["bv"], f)[:n_layers]
    wo = np.asarray(inputs["wo"], f)[:n_layers]
    bov = np.asarray(inputs["bo"], f)[:n_layers]
    ln2_s = np.asarray(inputs["ln2_s"], f)[:n_layers]
    ln2_b = np.asarray(inputs["ln2_b"], f)[:n_layers]
    w1 = np.asarray(inputs["w1"], f)[:n_layers]
    b1v = np.asarray(inputs["b1"], f)[:n_layers]
    w2 = np.asarray(inputs["w2"], f)[:n_layers]
    b2v = np.asarray(inputs["b2"], f)[:n_layers]
    lnf_s = np.asarray(inputs["lnf_s"], f)
    lnf_b = np.asarray(inputs["lnf_b"], f)

    sc = 1.0 / np.sqrt(HD)
    Wq = ln1_s[:, :, None] * wq * sc
    bq_f = (np.einsum("ld,ldm->lm", ln1_b, wq) + bqv) * sc
    Wk = ln1_s[:, :, None] * wk
    bk_f = np.einsum("ld,ldm->lm", ln1_b, wk) + bkv
    Wv = ln1_s[:, :, None] * wv
    bv_f = np.einsum("ld,ldm->lm", ln1_b, wv) + bvv
    bo_f = bov + np.einsum("lm,lmn->ln", bv_f, wo)
    W1 = ln2_s[:, :, None] * w1
    b1_f = np.einsum("ld,ldf->lf", ln2_b, w1) + b1v

    lw = n_layers

    def mtile(w, mt):
        kt = w.shape[1] // 128
        return np.ascontiguousarray(
            w.reshape(lw, kt, 128, mt, 128)
            .transpose(0, 3, 2, 1, 4)
            .reshape(lw, mt, 128, kt * 128)
        ).astype(bf)

    def bcol(b):
        return b.reshape(lw, DT, 128).transpose(0, 2, 1)

    bias = np.concatenate(
        [
            bcol(bq_f),
            bcol(bk_f),
            bcol(bo_f),
            bcol(b2v),
            b1_f.reshape(lw, FT, 128).transpose(0, 2, 1),
        ],
        axis=2,
    )

    arrs = {
        "wq": mtile(Wq, DT),
        "wk": mtile(Wk, DT),
        # wv packed kd-major along columns: [128, kd*D + c]
        "wv": np.ascontiguousarray(
            Wv.reshape(lw, DT, 128, D).transpose(0, 2, 1, 3).reshape(lw, 128, DT * D)
        ).astype(bf),
        "wo": mtile(wo, DT),
        "w1": mtile(W1, FT),
        "w2": mtile(w2, DT),
        "bias": np.ascontiguousarray(bias.astype(f)),
        "lnf": np.ascontiguousarray(
            np.stack([lnf_s.reshape(DT, 128).T, lnf_b.reshape(DT, 128).T], -1)
        ),
        "onesd": np.ones((128, S), f),
    }
    xT = np.ascontiguousarray(x.transpose(0, 2, 1))  # [B, D, S]
    return arrs, xT


def kernel(**inputs):
    from concourse.bass_utils import run_bass_kernel_spmd

    n_layers = L
    if "nc" not in _CACHE:
        _CACHE["nc"] = _build(n_layers)
    nc = _CACHE["nc"]
    arrs, xT = _prep(inputs, n_layers)
    in_maps = [dict(arrs, xT=xT[c]) for c in range(B)]
    res = run_bass_kernel_spmd(nc, in_maps, core_ids=list(range(B)))
    out = np.stack([res.results[c]["outT"].T for c in range(B)])
    return np.ascontiguousarray(out.astype(np.float32))
